# revision 34
# baseline (speedup 1.0000x reference)
"""Trainium2 Bass kernel for GQA causal attention (nn_Attention_83623013253180).

Shapes: B=2, L=2048, D=1024, H=16 heads, G=2 kv-groups, HPG=8, DQK=DV=128.

Sharding (8 cores): core c -> (b = c//4, g = (c%4)//2, hh = c%2), each core
handles one batch, one kv group, and 4 of that group's 8 query heads.
Wq/Wk/Wv are column-sharded, Wo row-sharded; the out-proj all-reduce (sum of
4 partials per batch) is done on host after gather, along with + bo.

Per-core device kernel.  Performance structure (~157-160us vs the 187us
fp16 baseline; rel err ~8.7e-3 vs the 2e-2 gate):
  - Q/K projections run as fp8-e4m3 DoubleRow matmuls: the D=1024
    contraction is packed in pairs of 128-tiles ([128, 2, N] operands), so
    each projection needs 4 double-pumped matmuls instead of 8.  Weights are
    pre-scaled by 256 on host (avoiding fp8 subnormals); the 1/65536 factor
    is folded into the softmax exp scale.  fp8 error washes out through the
    softmax; V/ctx/out paths stay fp16.
  - exp runs on kv-tile PAIRS ([128, 2, 512] PSUM score tiles) to halve the
    ScalarE per-instruction overhead (ScalarE is nearly 1:1 with PE in the
    attention phase); scores are written at their absolute q-offset so
    e-tile slicing stays uniform; dead columns hold exp(garbage) that is
    never streamed.
  - scores for head h+1 interleave with the ctx accumulation of head h so
    ScalarE always has a backlog; each ctx chain's normalize/transpose is
    emitted as soon as the chain stops (its diagonal pair), so every PSUM
    ring slot has its consumers emitted before reuse and the DVE rcp/norm
    runs ahead of the PE transposes.
  - the diagonal-mask DVE multiplies are DEFERRED to the consuming head's
    first iteration so the finalize rcp/norm ops (which gate PE transposes)
    are not stuck behind them in the DVE queue (-26us).
  - each chunk's out-projection is deferred and interleaved into the next
    chunk's score prologue; the last chunk's is emitted per q-tile as soon
    as that tile finalizes, via the then-idle score ring.  Output is
    stored/DMAed as fp16.
  - projections for chunk ch+1 are emitted after chunk ch's finalize.

PSUM budget (8 banks): sp tag [128,2,512]f32 x2 bufs (4 banks; score
pairs + K/Q projection accumulators) + cx tag x4 bufs (4 banks; ctx
accumulators [128,129]f32 one chain per bank — interleaving two
accumulation chains in one bank corrupts has_written state — plus V-proj,
transpose, and out-proj tiles rotating through the same ring).
"""

import numpy as np
import ml_dtypes

import concourse.bass as bass
import concourse.mybir as mybir
import concourse.tile as tile
from concourse import bacc
from concourse.bass_utils import run_bass_kernel_spmd

F8 = mybir.dt.float8e4
F16 = mybir.dt.float16
F32 = mybir.dt.float32
DR = mybir.MatmulPerfMode.DoubleRow

B, L, D = 2, 2048, 1024
H, G, HPG = 16, 2, 8
DQK = DV = 128
NHEAD = 4          # heads per core
NDT = D // 128     # 8 contraction tiles over input dim
NDTP = NDT // 2    # 4 DoubleRow tile-pairs
NKV = L // 128     # 16 kv tiles
QC = 512           # q chunk width
NQC = L // QC      # 4 q chunks
NCORES = 8


def _build(scale_val: float) -> bass.Bass:
    nc = bacc.Bacc("TRN2", target_bir_lowering=False, debug=False, num_devices=NCORES)

    xq8 = nc.dram_tensor("xq8", [NQC, 128, NDTP, 2, QC], F8, kind="ExternalInput")
    xk8 = nc.dram_tensor("xk8", [NQC, 128, NDTP, 2, QC], F8, kind="ExternalInput")
    xv = nc.dram_tensor("xvT", [NQC, 128, NDT, QC], F16, kind="ExternalInput")
    wq8 = nc.dram_tensor("wq8", [128, NDTP, 2, NHEAD * DQK], F8, kind="ExternalInput")
    wk8 = nc.dram_tensor("wk8", [128, NDTP, 2, DQK], F8, kind="ExternalInput")
    wv = nc.dram_tensor("wv", [128, NDT, DV + 1], F16, kind="ExternalInput")
    wo = nc.dram_tensor("wo", [128, NHEAD, D], F16, kind="ExternalInput")
    bq = nc.dram_tensor("bq", [128, NHEAD], F32, kind="ExternalInput")
    bk = nc.dram_tensor("bk", [128, 1], F32, kind="ExternalInput")
    bvb = nc.dram_tensor("bvb", [128, DV + 1], F32, kind="ExternalInput")
    msk = nc.dram_tensor("msk", [128, 128], F16, kind="ExternalInput")
    idn = nc.dram_tensor("idn", [128, 128], F16, kind="ExternalInput")
    out = nc.dram_tensor("out", [L, D], F16, kind="ExternalOutput")

    with tile.TileContext(nc) as tc:
        with (
            tc.tile_pool(name="const", bufs=1) as cpool,
            tc.tile_pool(name="xbuf", bufs=1) as xpool,
            tc.tile_pool(name="qkv", bufs=1) as qkvpool,
            tc.tile_pool(name="ebuf", bufs=18) as epool,
            tc.tile_pool(name="masked", bufs=8) as mpool,
            tc.tile_pool(name="wrk", bufs=16) as wpool,
            tc.tile_pool(name="ctxt", bufs=3) as ctpool,
            tc.tile_pool(name="outb", bufs=4) as opool,
            tc.tile_pool(name="ps_sp", bufs=2, space="PSUM") as ps_sp,
            tc.tile_pool(name="ps_cx", bufs=4, space="PSUM") as ps_cx,
        ):
            wq8_sb = cpool.tile([128, NDTP, 2, NHEAD * DQK], F8, tag="wq8")
            wk8_sb = cpool.tile([128, NDTP, 2, DQK], F8, tag="wk8")
            wv_sb = cpool.tile([128, NDT, DV + 1], F16, tag="wv")
            wo_sb = cpool.tile([128, NHEAD, D], F16, tag="wo")
            bq_sb = cpool.tile([128, NHEAD], F32, tag="bq")
            bk_sb = cpool.tile([128, 1], F32, tag="bk")
            bvb_sb = cpool.tile([128, DV + 1], F32, tag="bvb")
            msk_sb = cpool.tile([128, 128], F16, tag="msk")
            idn_sb = cpool.tile([128, 128], F16, tag="idn")

            q_sb = qkvpool.tile([128, NHEAD, L], F16, tag="q")     # qT per head
            k_sb = qkvpool.tile([128, L], F16, tag="k")            # kT
            v_sb = qkvpool.tile([128, NKV, DV + 1], F16, tag="v")  # V_aug tiles

            xq8_sb = xpool.tile([128, NQC, NDTP, 2, QC], F8, tag="xq8")
            xk8_sb = xpool.tile([128, NQC, NDTP, 2, QC], F8, tag="xk8")
            xv_sb = xpool.tile([128, NQC, NDT, QC], F16, tag="xv")

            def emit_kproj(ch):
                sl = slice(ch * QC, (ch + 1) * QC)
                pk = ps_cx.tile([128, QC], F32, tag="cx", name=f"pk_{ch}")
                for dtp in range(NDTP):
                    nc.tensor.matmul(
                        pk, wk8_sb[:, dtp], xk8_sb[:, ch, dtp],
                        start=(dtp == 0), stop=(dtp == NDTP - 1), perf_mode=DR,
                    )
                nc.vector.tensor_tensor(
                    k_sb[:, sl], pk, bk_sb[:].to_broadcast((128, QC)),
                    mybir.AluOpType.add,
                )

            def emit_qproj(ch):
                sl = slice(ch * QC, (ch + 1) * QC)
                for hi in range(NHEAD):
                    pq = ps_cx.tile([128, QC], F32, tag="cx", name=f"pq_{ch}_{hi}")
                    for dtp in range(NDTP):
                        nc.tensor.matmul(
                            pq,
                            wq8_sb[:, dtp, :, hi * DQK:(hi + 1) * DQK],
                            xq8_sb[:, ch, dtp],
                            start=(dtp == 0), stop=(dtp == NDTP - 1), perf_mode=DR,
                        )
                    nc.vector.tensor_tensor(
                        q_sb[:, hi, sl], pq,
                        bq_sb[:, hi:hi + 1].to_broadcast((128, QC)),
                        mybir.AluOpType.add,
                    )

            def emit_vproj(ch):
                for kvs in range(4):
                    kv = ch * 4 + kvs
                    pv = ps_cx.tile([128, DV + 1], F32, tag="cx",
                                    name=f"pv_{kv}")
                    for dt_i in range(NDT):
                        nc.tensor.matmul(
                            pv, xv_sb[:, ch, dt_i, kvs * 128:(kvs + 1) * 128],
                            wv_sb[:, dt_i, :],
                            start=(dt_i == 0), stop=(dt_i == NDT - 1),
                        )
                    nc.vector.tensor_tensor(
                        v_sb[:, kv, :], pv, bvb_sb[:], mybir.AluOpType.add
                    )

            def emit_proj(ch):
                emit_kproj(ch)
                emit_qproj(ch)
                emit_vproj(ch)

            # deferred out-projection state from the previous chunk
            pending = []   # list of closures: out-proj(j)

            for ch in range(NQC):
                qc = ch
                npair = (4 * qc + 4) // 2

                # ---- input DMA; first chunk also brings weights ----
                if ch == 0:
                    nc.sync.dma_start(xk8_sb[:, 0], xk8[0])
                    nc.sync.dma_start(wk8_sb[:], wk8[:])
                    nc.sync.dma_start(bk_sb[:], bk[:])
                    nc.sync.dma_start(xq8_sb[:, 0], xq8[0])
                    nc.sync.dma_start(wq8_sb[:], wq8[:])
                    nc.sync.dma_start(bq_sb[:], bq[:])
                    nc.sync.dma_start(msk_sb[:], msk[:])
                    nc.sync.dma_start(idn_sb[:], idn[:])
                    nc.sync.dma_start(xv_sb[:, 0], xv[0])
                    nc.sync.dma_start(wv_sb[:], wv[:])
                    nc.sync.dma_start(bvb_sb[:], bvb[:])
                    nc.sync.dma_start(wo_sb[:], wo[:])
                    emit_proj(0)
                if ch < NQC - 1:
                    nc.sync.dma_start(xk8_sb[:, ch + 1], xk8[ch + 1])
                    nc.sync.dma_start(xq8_sb[:, ch + 1], xq8[ch + 1])
                    nc.sync.dma_start(xv_sb[:, ch + 1], xv[ch + 1])

                # ---- attention for q chunk ch (kv tiles 0..4*qc+3) ----
                e_tiles = [[None] * npair for _ in range(NHEAD)]
                em_tiles = {}
                maskq = []   # deferred diag-mask DVE ops: (hi, t, e_t, i)
                ctx_ps = [None] * NHEAD

                def emit_scores(hi, p, qc=qc):
                    sp = ps_sp.tile([128, 2, QC], F32, tag="sp",
                                    name=f"sp_{qc}_{hi}_{p}")
                    t0 = 2 * p - 4 * qc
                    qoff_pair = max(t0, 0) * 128
                    for i in range(2):
                        kv = 2 * p + i
                        t = kv - 4 * qc
                        qoff = max(t, 0) * 128
                        nc.tensor.matmul(
                            sp[:, i, qoff:],
                            k_sb[:, kv * 128:(kv + 1) * 128],
                            q_sb[:, hi, qc * QC + qoff:(qc + 1) * QC],
                            start=True, stop=True,
                        )
                    e_t = epool.tile([128, 2, QC], F16, tag="e",
                                     name=f"e_{qc}_{hi}_{p}")
                    nc.scalar.activation(
                        e_t[:, :, qoff_pair:], sp[:, :, qoff_pair:],
                        mybir.ActivationFunctionType.Exp,
                        bias=0.0, scale=scale_val,
                    )
                    e_tiles[hi][p] = e_t
                    for i in range(2):
                        t = 2 * p + i - 4 * qc
                        if t >= 0:
                            if qc == 0:
                                emit_mask(hi, t, e_t, i)
                            else:
                                maskq.append((hi, t, e_t, i))

                def emit_mask(hi, t, e_t, i):
                    em = mpool.tile([128, 128], F16, tag="em",
                                    name=f"em_{qc}_{hi}_{t}")
                    nc.vector.tensor_tensor(
                        em[:], e_t[:, i, t * 128:(t + 1) * 128],
                        msk_sb[:], mybir.AluOpType.mult,
                    )
                    em_tiles[(hi, t)] = em

                def ctx_view(hi, j):
                    return ctx_ps[hi][j]

                def emit_ctx(hi, p, qc=qc):
                    if p == 0:
                        ctx_ps[hi] = [
                            ps_cx.tile([128, DV + 1], F32, tag="cx",
                                       name=f"ctx_{qc}_{hi}_{jj}")
                            for jj in range(4)
                        ]
                    e_t = e_tiles[hi][p]
                    for i in range(2):
                        kv = 2 * p + i
                        t = kv - 4 * qc
                        for j in range(4):
                            if kv > 4 * qc + j:
                                continue
                            if t == j and t >= 0:
                                e_use = em_tiles[(hi, t)][:]
                            else:
                                e_use = e_t[:, i, j * 128:(j + 1) * 128]
                            nc.tensor.matmul(
                                ctx_view(hi, j),
                                e_use,
                                v_sb[:, kv, :],
                                start=(kv == 0), stop=(kv == 4 * qc + j),
                            )

                ctxT = ctpool.tile([128, NHEAD, 4, 128], F16, tag="ctxT",
                                   name=f"ctxT_{qc}")

                def emit_fin(hi, j, ctxT=ctxT, ctx_ps=ctx_ps):
                    rcp = wpool.tile([128, 1], F32, tag="rcp")
                    cva = ctx_ps[hi][j]
                    nc.vector.reciprocal(rcp[:], cva[:, DV:DV + 1])
                    ctxn = wpool.tile([128, 128], F16, tag="ctxn")
                    nc.vector.tensor_tensor(
                        ctxn[:], cva[:, 0:DV],
                        rcp[:].to_broadcast((128, DV)),
                        mybir.AluOpType.mult,
                    )
                    tr_ps = ps_cx.tile([128, 128], F16, tag="cx",
                                       name=f"tr_{qc}_{hi}_{j}")
                    nc.tensor.transpose(tr_ps, ctxn[:], idn_sb[:])
                    nc.vector.tensor_copy(ctxT[:, hi, j, :], tr_ps)

                def make_out_task(j, qc=qc, ctxT=ctxT):
                    def task():
                        o_sb = opool.tile([128, 2, QC], F16, tag="o")
                        for nch in range(2):
                            po = ps_cx.tile([128, QC], F32, tag="cx",
                                            name=f"po_{qc}_{j}_{nch}")
                            for hi in range(NHEAD):
                                nc.tensor.matmul(
                                    po,
                                    ctxT[:, hi, j, :],
                                    wo_sb[:, hi, nch * 512:(nch + 1) * 512],
                                    start=(hi == 0), stop=(hi == NHEAD - 1),
                                )
                            nc.vector.tensor_copy(o_sb[:, nch, :], po)
                        qt = qc * 4 + j
                        nc.sync.dma_start(
                            out[qt * 128:(qt + 1) * 128, :], o_sb[:]
                        )
                    return task

                # ---- prologue: head-0 scores interleaved with the previous
                # chunk's deferred finalize(3)+out-projection ----
                for p in range(npair):
                    emit_scores(0, p)
                    if pending:
                        pending.pop(0)()
                while pending:
                    pending.pop(0)()

                def emit_out_sp(j, qc=qc, ctxT=ctxT):
                    # last-chunk out-proj via the (now idle) score ring
                    po = ps_sp.tile([128, 2, QC], F32, tag="sp",
                                    name=f"pos_{qc}_{j}")
                    o_sb = opool.tile([128, 2, QC], F16, tag="o")
                    for nch in range(2):
                        for hi in range(NHEAD):
                            nc.tensor.matmul(
                                po[:, nch, :],
                                ctxT[:, hi, j, :],
                                wo_sb[:, hi, nch * 512:(nch + 1) * 512],
                                start=(hi == 0), stop=(hi == NHEAD - 1),
                            )
                    nc.vector.tensor_copy(o_sb[:], po[:])
                    qt = qc * 4 + j
                    nc.sync.dma_start(out[qt * 128:(qt + 1) * 128, :], o_sb[:])

                # ---- main: ctx of head h with score lookahead of head h+1.
                # each ctx chain is finalized as soon as it stops (its
                # diagonal pair), so the cx-ring handoff always has its
                # consumers emitted before the slot is reused and the DVE
                # rcp/norm runs ahead of the PE transposes. ----
                for hi in range(NHEAD):
                    for p in range(npair):
                        if hi + 1 < NHEAD:
                            emit_scores(hi + 1, p)
                        if p == 0:
                            for it in [m for m in maskq if m[0] == hi]:
                                maskq.remove(it)
                                emit_mask(*it)
                        emit_ctx(hi, p)
                        for j in range(4):
                            if 2 * qc + j // 2 == p:
                                emit_fin(hi, j)
                                if hi == NHEAD - 1 and ch == NQC - 1:
                                    emit_out_sp(j)
                if ch < NQC - 1:
                    emit_proj(ch + 1)
                    # defer the out-projection into the next chunk's prologue
                    pending = [make_out_task(j) for j in range(4)]
                else:
                    pending = []

            # ---- flush the last chunk's finalize + out-projection ----
            while pending:
                pending.pop(0)()

    nc.finalize()
    return nc


_NC_CACHE: dict[float, bass.Bass] = {}


def _get_nc(scale_val: float) -> bass.Bass:
    if scale_val not in _NC_CACHE:
        _NC_CACHE[scale_val] = _build(scale_val)
    return _NC_CACHE[scale_val]


def _chunk_tile(a: np.ndarray) -> np.ndarray:
    """[K, F] -> [F//QC, 128, K//128, QC] chunk-major partition-tiled fp16."""
    k, f = a.shape
    b = a.reshape(k // 128, 128, f // QC, QC)          # [po, pi, ch, qc]
    return np.ascontiguousarray(
        b.transpose(2, 1, 0, 3).astype(np.float16)     # [ch, pi, po, qc]
    )


def _chunk_tile_f8(a: np.ndarray) -> np.ndarray:
    """[K, F] -> [F//QC, 128, K//256, 2, QC] DoubleRow-interleaved fp8."""
    k, f = a.shape
    b = a.reshape(k // 256, 2, 128, f // QC, QC)       # [dtp, j, p, ch, qc]
    return np.ascontiguousarray(
        b.transpose(3, 2, 0, 1, 4).astype(ml_dtypes.float8_e4m3)
    )


def _part_tile(a: np.ndarray) -> np.ndarray:
    """[K, F] -> [128, K//128, F] partition-tiled fp16 contiguous."""
    k, f = a.shape
    return np.ascontiguousarray(
        a.reshape(k // 128, 128, f).transpose(1, 0, 2).astype(np.float16)
    )


def _part_tile_f8(a: np.ndarray) -> np.ndarray:
    """[K, F] -> [128, K//256, 2, F] DoubleRow-interleaved fp8."""
    k, f = a.shape
    return np.ascontiguousarray(
        a.reshape(k // 256, 2, 128, f).transpose(2, 0, 1, 3)
        .astype(ml_dtypes.float8_e4m3)
    )


W8SCALE = 256.0


def run(inputs: dict, trace: bool = False):
    in_q = np.asarray(inputs["in_q"], np.float32)
    in_k = np.asarray(inputs["in_k"], np.float32)
    in_v = np.asarray(inputs["in_v"], np.float32)
    Wq = np.asarray(inputs["Wq"], np.float32)
    Wk = np.asarray(inputs["Wk"], np.float32)
    Wv = np.asarray(inputs["Wv"], np.float32)
    Wo = np.asarray(inputs["Wo"], np.float32)
    bq = np.asarray(inputs["bq"], np.float32)
    bk = np.asarray(inputs["bk"], np.float32)
    bv = np.asarray(inputs["bv"], np.float32)
    bo = np.asarray(inputs["bo"], np.float32)
    qes = float(np.asarray(inputs["q_extra_scale"], np.float32).reshape(-1)[0])

    scale_val = qes / float(np.sqrt(DQK)) / (W8SCALE * W8SCALE)
    nc = _get_nc(scale_val)

    # triangular mask for the single diagonal 128x128 block
    ii = np.arange(128)[:, None]
    jj = np.arange(128)[None, :]
    masks = (jj >= ii).astype(np.float16)  # [128, 128], 1 where q >= kv
    idn = np.eye(128, dtype=np.float16)

    in_maps = []
    for c in range(NCORES):
        b, g, hh = c // 4, (c % 4) // 2, c % 2
        h0 = g * HPG + hh * NHEAD
        wv_aug = np.concatenate(
            [Wv[:, g * DV:(g + 1) * DV], np.zeros((D, 1), np.float32)], axis=1
        )
        bv_aug = np.concatenate([bv[g * DV:(g + 1) * DV], [1.0]]).astype(np.float32)
        wo_slice = Wo[h0 * DV:(h0 + NHEAD) * DV, :]  # [512, 1024]
        in_maps.append({
            "xq8": _chunk_tile_f8(in_q[b].T),
            "xk8": _chunk_tile_f8(in_k[b].T),
            "xvT": _chunk_tile(in_v[b].T),
            "wq8": _part_tile_f8(Wq[:, h0 * DQK:(h0 + NHEAD) * DQK] * W8SCALE),
            "wk8": _part_tile_f8(Wk[:, g * DQK:(g + 1) * DQK] * W8SCALE),
            "wv": _part_tile(wv_aug),
            "wo": np.ascontiguousarray(
                wo_slice.reshape(NHEAD, DV, D).transpose(1, 0, 2).astype(np.float16)
            ),
            "bq": np.ascontiguousarray(
                (bq[h0 * DQK:(h0 + NHEAD) * DQK] * W8SCALE)
                .reshape(NHEAD, DQK).T.astype(np.float32)
            ),
            "bk": (bk[g * DQK:(g + 1) * DQK] * W8SCALE)
            .reshape(DQK, 1).astype(np.float32),
            "bvb": np.ascontiguousarray(
                np.broadcast_to(bv_aug, (128, DV + 1)).astype(np.float32)
            ),
            "msk": masks,
            "idn": idn,
        })

    res = run_bass_kernel_spmd(
        nc, in_maps, core_ids=list(range(NCORES)), trace=trace
    )

    out_full = np.zeros((B, L, D), np.float32)
    for c in range(NCORES):
        out_full[c // 4] += np.asarray(res.results[c]["out"], np.float32)
    out_full += bo
    return out_full, res.exec_time_ns


def kernel(**inputs) -> np.ndarray:
    out, _ = run(inputs, trace=False)
    return out


# revision 35
# speedup vs baseline: 1.0427x; 1.0427x over previous
"""Trainium2 Bass kernel for GQA causal attention (nn_Attention_83623013253180).

Shapes: B=2, L=2048, D=1024, H=16 heads, G=2 kv-groups, HPG=8, DQK=DV=128.

Sharding (8 cores): core c -> (b = c//4, g = (c%4)//2, hh = c%2), each core
handles one batch, one kv group, and 4 of that group's 8 query heads.
Wq/Wk/Wv are column-sharded, Wo row-sharded; the out-proj all-reduce (sum of
4 partials per batch) is done on host after gather, along with + bo.

Per-core device kernel.  Performance structure (~157-160us vs the 187us
fp16 baseline; rel err ~8.7e-3 vs the 2e-2 gate):
  - Q/K projections run as fp8-e4m3 DoubleRow matmuls: the D=1024
    contraction is packed in pairs of 128-tiles ([128, 2, N] operands), so
    each projection needs 4 double-pumped matmuls instead of 8.  Weights are
    pre-scaled by 256 on host (avoiding fp8 subnormals); the 1/65536 factor
    is folded into the softmax exp scale.  fp8 error washes out through the
    softmax; V/ctx/out paths stay fp16.
  - exp runs on kv-tile PAIRS ([128, 2, 512] PSUM score tiles) to halve the
    ScalarE per-instruction overhead (ScalarE is nearly 1:1 with PE in the
    attention phase); scores are written at their absolute q-offset so
    e-tile slicing stays uniform; dead columns hold exp(garbage) that is
    never streamed.
  - scores for head h+1 interleave with the ctx accumulation of head h so
    ScalarE always has a backlog; each ctx chain's normalize/transpose is
    emitted as soon as the chain stops (its diagonal pair), so every PSUM
    ring slot has its consumers emitted before reuse and the DVE rcp/norm
    runs ahead of the PE transposes.
  - the diagonal-mask DVE multiplies are DEFERRED to the consuming head's
    first iteration so the finalize rcp/norm ops (which gate PE transposes)
    are not stuck behind them in the DVE queue (-26us).
  - each chunk's out-projection is deferred and interleaved into the next
    chunk's score prologue; the last chunk's is emitted per q-tile as soon
    as that tile finalizes, via the then-idle score ring.  Output is
    stored/DMAed as fp16.
  - projections for chunk ch+1 are emitted after chunk ch's finalize.

PSUM budget (8 banks): sp tag [128,2,512]f32 x2 bufs (4 banks; score
pairs + K/Q projection accumulators) + cx tag x4 bufs (4 banks; ctx
accumulators [128,129]f32 one chain per bank — interleaving two
accumulation chains in one bank corrupts has_written state — plus V-proj,
transpose, and out-proj tiles rotating through the same ring).
"""

import numpy as np
import ml_dtypes

import concourse.bass as bass
import concourse.mybir as mybir
import concourse.tile as tile
from concourse import bacc
from concourse.bass_utils import run_bass_kernel_spmd

F8 = mybir.dt.float8e4
F16 = mybir.dt.float16
F32 = mybir.dt.float32
DR = mybir.MatmulPerfMode.DoubleRow

B, L, D = 2, 2048, 1024
H, G, HPG = 16, 2, 8
DQK = DV = 128
NHEAD = 4          # heads per core
NDT = D // 128     # 8 contraction tiles over input dim
NDTP = NDT // 2    # 4 DoubleRow tile-pairs
NKV = L // 128     # 16 kv tiles
QC = 512           # q chunk width
NQC = L // QC      # 4 q chunks
NCORES = 8


def _build(scale_val: float) -> bass.Bass:
    nc = bacc.Bacc("TRN2", target_bir_lowering=False, debug=False, num_devices=NCORES)

    xq8 = nc.dram_tensor("xq8", [NQC, 128, NDTP, 2, QC], F8, kind="ExternalInput")
    xk8 = nc.dram_tensor("xk8", [NQC, 128, NDTP, 2, QC], F8, kind="ExternalInput")
    xv = nc.dram_tensor("xvT", [NQC, 128, NDT, QC], F16, kind="ExternalInput")
    wq8 = nc.dram_tensor("wq8", [128, NDTP, 2, NHEAD * DQK], F8, kind="ExternalInput")
    wk8 = nc.dram_tensor("wk8", [128, NDTP, 2, DQK], F8, kind="ExternalInput")
    wv = nc.dram_tensor("wv", [128, NDT, DV + 1], F16, kind="ExternalInput")
    wo = nc.dram_tensor("wo", [128, NHEAD, D], F16, kind="ExternalInput")
    bq = nc.dram_tensor("bq", [128, NHEAD], F32, kind="ExternalInput")
    bk = nc.dram_tensor("bk", [128, 1], F32, kind="ExternalInput")
    bvb = nc.dram_tensor("bvb", [128, DV + 1], F32, kind="ExternalInput")
    msk = nc.dram_tensor("msk", [128, 128], F16, kind="ExternalInput")
    idn = nc.dram_tensor("idn", [128, 128], F16, kind="ExternalInput")
    out = nc.dram_tensor("out", [L, D], F16, kind="ExternalOutput")

    with tile.TileContext(nc) as tc:
        with (
            tc.tile_pool(name="const", bufs=1) as cpool,
            tc.tile_pool(name="xbuf", bufs=1) as xpool,
            tc.tile_pool(name="qkv", bufs=1) as qkvpool,
            tc.tile_pool(name="ebuf", bufs=18) as epool,
            tc.tile_pool(name="masked", bufs=8) as mpool,
            tc.tile_pool(name="wrk", bufs=16) as wpool,
            tc.tile_pool(name="ctxt", bufs=3) as ctpool,
            tc.tile_pool(name="outb", bufs=4) as opool,
            tc.tile_pool(name="ps_sp", bufs=2, space="PSUM") as ps_sp,
            tc.tile_pool(name="ps_cx", bufs=4, space="PSUM") as ps_cx,
        ):
            wq8_sb = cpool.tile([128, NDTP, 2, NHEAD * DQK], F8, tag="wq8")
            wk8_sb = cpool.tile([128, NDTP, 2, DQK], F8, tag="wk8")
            wv_sb = cpool.tile([128, NDT, DV + 1], F16, tag="wv")
            wo_sb = cpool.tile([128, NHEAD, D], F16, tag="wo")
            bq_sb = cpool.tile([128, NHEAD], F32, tag="bq")
            bk_sb = cpool.tile([128, 1], F32, tag="bk")
            bvb_sb = cpool.tile([128, DV + 1], F32, tag="bvb")
            msk_sb = cpool.tile([128, 128], F16, tag="msk")
            idn_sb = cpool.tile([128, 128], F16, tag="idn")

            q_sb = qkvpool.tile([128, NHEAD, L], F16, tag="q")     # qT per head
            k_sb = qkvpool.tile([128, L], F16, tag="k")            # kT
            v_sb = qkvpool.tile([128, NKV, DV + 1], F16, tag="v")  # V_aug tiles

            xq8_sb = xpool.tile([128, NQC, NDTP, 2, QC], F8, tag="xq8")
            xk8_sb = xpool.tile([128, NQC, NDTP, 2, QC], F8, tag="xk8")
            xv_sb = xpool.tile([128, NQC, NDT, QC], F16, tag="xv")

            def emit_kproj(ch):
                sl = slice(ch * QC, (ch + 1) * QC)
                pk = ps_sp.tile([128, 2, QC], F32, tag="sp", name=f"pk_{ch}")
                for dtp in range(NDTP):
                    nc.tensor.matmul(
                        pk[:, 0, :], wk8_sb[:, dtp], xk8_sb[:, ch, dtp],
                        start=(dtp == 0), stop=(dtp == NDTP - 1), perf_mode=DR,
                    )
                nc.vector.tensor_tensor(
                    k_sb[:, sl], pk[:, 0, :], bk_sb[:].to_broadcast((128, QC)),
                    mybir.AluOpType.add,
                )

            def emit_qproj(ch):
                sl = slice(ch * QC, (ch + 1) * QC)
                for hi in range(NHEAD):
                    pq = ps_sp.tile([128, 2, QC], F32, tag="sp", name=f"pq_{ch}_{hi}")
                    for dtp in range(NDTP):
                        nc.tensor.matmul(
                            pq[:, 0, :],
                            wq8_sb[:, dtp, :, hi * DQK:(hi + 1) * DQK],
                            xq8_sb[:, ch, dtp],
                            start=(dtp == 0), stop=(dtp == NDTP - 1), perf_mode=DR,
                        )
                    nc.vector.tensor_tensor(
                        q_sb[:, hi, sl], pq[:, 0, :],
                        bq_sb[:, hi:hi + 1].to_broadcast((128, QC)),
                        mybir.AluOpType.add,
                    )

            def emit_vproj(ch):
                for kvs in range(4):
                    kv = ch * 4 + kvs
                    pv = ps_cx.tile([128, DV + 1], F32, tag="cx",
                                    name=f"pv_{kv}")
                    for dt_i in range(NDT):
                        nc.tensor.matmul(
                            pv, xv_sb[:, ch, dt_i, kvs * 128:(kvs + 1) * 128],
                            wv_sb[:, dt_i, :],
                            start=(dt_i == 0), stop=(dt_i == NDT - 1),
                        )
                    nc.vector.tensor_tensor(
                        v_sb[:, kv, :], pv, bvb_sb[:], mybir.AluOpType.add
                    )

            def emit_proj(ch):
                emit_kproj(ch)
                emit_qproj(ch)
                emit_vproj(ch)

            # deferred out-projection state from the previous chunk
            pending = []   # list of closures: out-proj(j)

            for ch in range(NQC):
                qc = ch
                npair = (4 * qc + 4) // 2

                # ---- input DMA; first chunk also brings weights ----
                if ch == 0:
                    nc.sync.dma_start(xk8_sb[:, 0], xk8[0])
                    nc.sync.dma_start(wk8_sb[:], wk8[:])
                    nc.sync.dma_start(bk_sb[:], bk[:])
                    nc.sync.dma_start(xq8_sb[:, 0], xq8[0])
                    nc.sync.dma_start(wq8_sb[:], wq8[:])
                    nc.sync.dma_start(bq_sb[:], bq[:])
                    nc.sync.dma_start(msk_sb[:], msk[:])
                    nc.sync.dma_start(idn_sb[:], idn[:])
                    nc.sync.dma_start(xv_sb[:, 0], xv[0])
                    nc.sync.dma_start(wv_sb[:], wv[:])
                    nc.sync.dma_start(bvb_sb[:], bvb[:])
                    nc.sync.dma_start(wo_sb[:], wo[:])
                    emit_proj(0)
                if ch < NQC - 1:
                    nc.sync.dma_start(xk8_sb[:, ch + 1], xk8[ch + 1])
                    nc.sync.dma_start(xq8_sb[:, ch + 1], xq8[ch + 1])
                    nc.sync.dma_start(xv_sb[:, ch + 1], xv[ch + 1])

                # ---- attention for q chunk ch (kv tiles 0..4*qc+3) ----
                e_tiles = [[None] * npair for _ in range(NHEAD)]
                em_tiles = {}
                maskq = []   # deferred diag-mask DVE ops: (hi, t, e_t, i)
                ctx_ps = [None] * NHEAD

                def emit_scores(hi, p, qc=qc):
                    sp = ps_sp.tile([128, 2, QC], F32, tag="sp",
                                    name=f"sp_{qc}_{hi}_{p}")
                    t0 = 2 * p - 4 * qc
                    qoff_pair = max(t0, 0) * 128
                    for i in range(2):
                        kv = 2 * p + i
                        t = kv - 4 * qc
                        qoff = max(t, 0) * 128
                        nc.tensor.matmul(
                            sp[:, i, qoff:],
                            k_sb[:, kv * 128:(kv + 1) * 128],
                            q_sb[:, hi, qc * QC + qoff:(qc + 1) * QC],
                            start=True, stop=True,
                        )
                    e_t = epool.tile([128, 2, QC], F16, tag="e",
                                     name=f"e_{qc}_{hi}_{p}")
                    nc.scalar.activation(
                        e_t[:, :, qoff_pair:], sp[:, :, qoff_pair:],
                        mybir.ActivationFunctionType.Exp,
                        bias=0.0, scale=scale_val,
                    )
                    e_tiles[hi][p] = e_t
                    for i in range(2):
                        t = 2 * p + i - 4 * qc
                        if t >= 0:
                            if qc == 0:
                                emit_mask(hi, t, e_t, i)
                            else:
                                maskq.append((hi, t, e_t, i))

                def emit_mask(hi, t, e_t, i):
                    em = mpool.tile([128, 128], F16, tag="em",
                                    name=f"em_{qc}_{hi}_{t}")
                    nc.vector.tensor_tensor(
                        em[:], e_t[:, i, t * 128:(t + 1) * 128],
                        msk_sb[:], mybir.AluOpType.mult,
                    )
                    em_tiles[(hi, t)] = em

                def ctx_view(hi, j):
                    return ctx_ps[hi][j]

                def emit_ctx(hi, p, qc=qc):
                    if p == 0:
                        ctx_ps[hi] = [
                            ps_cx.tile([128, DV + 1], F32, tag="cx",
                                       name=f"ctx_{qc}_{hi}_{jj}")
                            for jj in range(4)
                        ]
                    e_t = e_tiles[hi][p]
                    for i in range(2):
                        kv = 2 * p + i
                        t = kv - 4 * qc
                        for j in range(4):
                            if kv > 4 * qc + j:
                                continue
                            if t == j and t >= 0:
                                e_use = em_tiles[(hi, t)][:]
                            else:
                                e_use = e_t[:, i, j * 128:(j + 1) * 128]
                            nc.tensor.matmul(
                                ctx_view(hi, j),
                                e_use,
                                v_sb[:, kv, :],
                                start=(kv == 0), stop=(kv == 4 * qc + j),
                            )

                ctxT = ctpool.tile([128, NHEAD, 4, 128], F16, tag="ctxT",
                                   name=f"ctxT_{qc}")

                def emit_fin(hi, j, ctxT=ctxT, ctx_ps=ctx_ps):
                    rcp = wpool.tile([128, 1], F32, tag="rcp")
                    cva = ctx_ps[hi][j]
                    nc.vector.reciprocal(rcp[:], cva[:, DV:DV + 1])
                    ctxn = wpool.tile([128, 128], F16, tag="ctxn")
                    nc.vector.tensor_tensor(
                        ctxn[:], cva[:, 0:DV],
                        rcp[:].to_broadcast((128, DV)),
                        mybir.AluOpType.mult,
                    )
                    tr_ps = ps_cx.tile([128, 128], F16, tag="cx",
                                       name=f"tr_{qc}_{hi}_{j}")
                    nc.tensor.transpose(tr_ps, ctxn[:], idn_sb[:])
                    nc.vector.tensor_copy(ctxT[:, hi, j, :], tr_ps)

                def make_out_task(j, qc=qc, ctxT=ctxT):
                    def task():
                        o_sb = opool.tile([128, 2, QC], F16, tag="o")
                        for nch in range(2):
                            po = ps_cx.tile([128, QC], F32, tag="cx",
                                            name=f"po_{qc}_{j}_{nch}")
                            for hi in range(NHEAD):
                                nc.tensor.matmul(
                                    po,
                                    ctxT[:, hi, j, :],
                                    wo_sb[:, hi, nch * 512:(nch + 1) * 512],
                                    start=(hi == 0), stop=(hi == NHEAD - 1),
                                )
                            nc.vector.tensor_copy(o_sb[:, nch, :], po)
                        qt = qc * 4 + j
                        nc.sync.dma_start(
                            out[qt * 128:(qt + 1) * 128, :], o_sb[:]
                        )
                    return task

                # ---- prologue: head-0 scores interleaved with the previous
                # chunk's deferred finalize(3)+out-projection ----
                for p in range(npair):
                    emit_scores(0, p)
                    if pending:
                        pending.pop(0)()
                while pending:
                    pending.pop(0)()

                def emit_out_sp(j, qc=qc, ctxT=ctxT):
                    # last-chunk out-proj via the (now idle) score ring
                    po = ps_sp.tile([128, 2, QC], F32, tag="sp",
                                    name=f"pos_{qc}_{j}")
                    o_sb = opool.tile([128, 2, QC], F16, tag="o")
                    for nch in range(2):
                        for hi in range(NHEAD):
                            nc.tensor.matmul(
                                po[:, nch, :],
                                ctxT[:, hi, j, :],
                                wo_sb[:, hi, nch * 512:(nch + 1) * 512],
                                start=(hi == 0), stop=(hi == NHEAD - 1),
                            )
                    nc.vector.tensor_copy(o_sb[:], po[:])
                    qt = qc * 4 + j
                    nc.sync.dma_start(out[qt * 128:(qt + 1) * 128, :], o_sb[:])

                # ---- main: ctx of head h with score lookahead of head h+1.
                # each ctx chain is finalized as soon as it stops (its
                # diagonal pair), so the cx-ring handoff always has its
                # consumers emitted before the slot is reused and the DVE
                # rcp/norm runs ahead of the PE transposes. ----
                for hi in range(NHEAD):
                    for p in range(npair):
                        if hi + 1 < NHEAD:
                            emit_scores(hi + 1, p)
                        if p == 0:
                            for it in [m for m in maskq if m[0] == hi]:
                                maskq.remove(it)
                                emit_mask(*it)
                        emit_ctx(hi, p)
                        for j in range(4):
                            if 2 * qc + j // 2 == p:
                                emit_fin(hi, j)
                                if hi == NHEAD - 1 and ch == NQC - 1:
                                    emit_out_sp(j)
                if ch < NQC - 1:
                    emit_proj(ch + 1)
                    # defer the out-projection into the next chunk's prologue
                    pending = [make_out_task(j) for j in range(4)]
                else:
                    pending = []

            # ---- flush the last chunk's finalize + out-projection ----
            while pending:
                pending.pop(0)()

    nc.finalize()
    return nc


_NC_CACHE: dict[float, bass.Bass] = {}


def _get_nc(scale_val: float) -> bass.Bass:
    if scale_val not in _NC_CACHE:
        _NC_CACHE[scale_val] = _build(scale_val)
    return _NC_CACHE[scale_val]


def _chunk_tile(a: np.ndarray) -> np.ndarray:
    """[K, F] -> [F//QC, 128, K//128, QC] chunk-major partition-tiled fp16."""
    k, f = a.shape
    b = a.reshape(k // 128, 128, f // QC, QC)          # [po, pi, ch, qc]
    return np.ascontiguousarray(
        b.transpose(2, 1, 0, 3).astype(np.float16)     # [ch, pi, po, qc]
    )


def _chunk_tile_f8(a: np.ndarray) -> np.ndarray:
    """[K, F] -> [F//QC, 128, K//256, 2, QC] DoubleRow-interleaved fp8."""
    k, f = a.shape
    b = a.reshape(k // 256, 2, 128, f // QC, QC)       # [dtp, j, p, ch, qc]
    return np.ascontiguousarray(
        b.transpose(3, 2, 0, 1, 4).astype(ml_dtypes.float8_e4m3)
    )


def _part_tile(a: np.ndarray) -> np.ndarray:
    """[K, F] -> [128, K//128, F] partition-tiled fp16 contiguous."""
    k, f = a.shape
    return np.ascontiguousarray(
        a.reshape(k // 128, 128, f).transpose(1, 0, 2).astype(np.float16)
    )


def _part_tile_f8(a: np.ndarray) -> np.ndarray:
    """[K, F] -> [128, K//256, 2, F] DoubleRow-interleaved fp8."""
    k, f = a.shape
    return np.ascontiguousarray(
        a.reshape(k // 256, 2, 128, f).transpose(2, 0, 1, 3)
        .astype(ml_dtypes.float8_e4m3)
    )


W8SCALE = 256.0


def run(inputs: dict, trace: bool = False):
    in_q = np.asarray(inputs["in_q"], np.float32)
    in_k = np.asarray(inputs["in_k"], np.float32)
    in_v = np.asarray(inputs["in_v"], np.float32)
    Wq = np.asarray(inputs["Wq"], np.float32)
    Wk = np.asarray(inputs["Wk"], np.float32)
    Wv = np.asarray(inputs["Wv"], np.float32)
    Wo = np.asarray(inputs["Wo"], np.float32)
    bq = np.asarray(inputs["bq"], np.float32)
    bk = np.asarray(inputs["bk"], np.float32)
    bv = np.asarray(inputs["bv"], np.float32)
    bo = np.asarray(inputs["bo"], np.float32)
    qes = float(np.asarray(inputs["q_extra_scale"], np.float32).reshape(-1)[0])

    scale_val = qes / float(np.sqrt(DQK)) / (W8SCALE * W8SCALE)
    nc = _get_nc(scale_val)

    # triangular mask for the single diagonal 128x128 block
    ii = np.arange(128)[:, None]
    jj = np.arange(128)[None, :]
    masks = (jj >= ii).astype(np.float16)  # [128, 128], 1 where q >= kv
    idn = np.eye(128, dtype=np.float16)

    in_maps = []
    for c in range(NCORES):
        b, g, hh = c // 4, (c % 4) // 2, c % 2
        h0 = g * HPG + hh * NHEAD
        wv_aug = np.concatenate(
            [Wv[:, g * DV:(g + 1) * DV], np.zeros((D, 1), np.float32)], axis=1
        )
        bv_aug = np.concatenate([bv[g * DV:(g + 1) * DV], [1.0]]).astype(np.float32)
        wo_slice = Wo[h0 * DV:(h0 + NHEAD) * DV, :]  # [512, 1024]
        in_maps.append({
            "xq8": _chunk_tile_f8(in_q[b].T),
            "xk8": _chunk_tile_f8(in_k[b].T),
            "xvT": _chunk_tile(in_v[b].T),
            "wq8": _part_tile_f8(Wq[:, h0 * DQK:(h0 + NHEAD) * DQK] * W8SCALE),
            "wk8": _part_tile_f8(Wk[:, g * DQK:(g + 1) * DQK] * W8SCALE),
            "wv": _part_tile(wv_aug),
            "wo": np.ascontiguousarray(
                wo_slice.reshape(NHEAD, DV, D).transpose(1, 0, 2).astype(np.float16)
            ),
            "bq": np.ascontiguousarray(
                (bq[h0 * DQK:(h0 + NHEAD) * DQK] * W8SCALE)
                .reshape(NHEAD, DQK).T.astype(np.float32)
            ),
            "bk": (bk[g * DQK:(g + 1) * DQK] * W8SCALE)
            .reshape(DQK, 1).astype(np.float32),
            "bvb": np.ascontiguousarray(
                np.broadcast_to(bv_aug, (128, DV + 1)).astype(np.float32)
            ),
            "msk": masks,
            "idn": idn,
        })

    res = run_bass_kernel_spmd(
        nc, in_maps, core_ids=list(range(NCORES)), trace=trace
    )

    out_full = np.zeros((B, L, D), np.float32)
    for c in range(NCORES):
        out_full[c // 4] += np.asarray(res.results[c]["out"], np.float32)
    out_full += bo
    return out_full, res.exec_time_ns


def kernel(**inputs) -> np.ndarray:
    out, _ = run(inputs, trace=False)
    return out
